# revision 22
# baseline (speedup 1.0000x reference)
"""GAT (2-layer, PyG-style) forward on 8 TRN2 NeuronCores.

Sharding: dst-node blocks across cores; per-core edge lists routed/sorted by
dst block on host. Dense phase (replicated on every core) writes a node table
of 512-byte rows [h_bf16 | h_fp8 | a_src.h | a_dst.h] to DRAM (h features in
(c,h)-major order, head index fastest). Edge phase gathers per-edge payloads
with batched SWDGE dma_gather (two per dst block: lo/hi table halves, since
gather indices are int16), broadcasts a_dst[dst] onto edge lanes via a K=1
ones-matmul + is_equal selection, and does segment softmax + weighted sums as
selection-matrix matmuls in PSUM."""
import sys
if '/opt/trn_rl_repo' not in sys.path:
    sys.path.insert(0, '/opt/trn_rl_repo')
import json
import numpy as np
import ml_dtypes

import concourse.bass as bass
import concourse.mybir as mybir
import concourse.tile as tile
from concourse import library_config

bf16 = ml_dtypes.bfloat16
f8e4 = ml_dtypes.float8_e4m3fn
F32 = mybir.dt.float32
BF16 = mybir.dt.bfloat16
F8 = mybir.dt.float8e4
U8 = mybir.dt.uint8
I16 = mybir.dt.int16
I32 = mybir.dt.int32
ALU = mybir.AluOpType
ACTF = mybir.ActivationFunctionType

N_NODES = 50000
N_PAD = 50048            # multiple of 128; table rows (zero-padded)
N_CORES = 8
CORE_ROWS = N_NODES // N_CORES
SPLIT = 32768            # int16 gather index limit
ROWB = 512               # table row bytes


def mkap(ap, dims, elem_offset=0):
    """AP with explicit [step, count] free dims (elements) after the partition dim."""
    return bass.AP(ap.tensor, ap.offset + elem_offset,
                   [list(ap.ap[0])] + [list(d) for d in dims])


def build_gat_layer(FIN, H, C, NBF, T_lo, T_hi, mode, with_bias, ablate=()):
    """mode: 'elu' (layer 1, out = ELU(u)+1 in bf16) or 'mean_lsm' (layer 2).
    Table row: [NBF bf16 | NF8 fp8 | as 8xbf16 | ad 8xbf16], (c,h)-major h."""
    FOUT = H * C
    NF8 = FOUT - NBF
    TCOLS = FOUT + 2 * H          # dense psum: [h_bf | al_src | al_dst | h_f8]
    MCOLS = FOUT + H              # m' row: [ee*h_bf | ee*h_f8 | ee]
    assert 2 * NBF + 32 + NF8 <= ROWB and NBF % H == 0 and NF8 % H == 0
    AS2 = NBF                     # as offset in bf16 elems
    F8B = 2 * NBF + 32            # byte offset of f8 h-part
    ROWB2 = ROWB // 2
    n_blocks = len(T_lo)
    T_all = [int(T_lo[b] + T_hi[b]) for b in range(n_blocks)]
    toff = np.concatenate([[0], np.cumsum(T_all)]).astype(int)
    NT = int(toff[-1])
    KCH = FIN // 128
    last_blk_rows = CORE_ROWS - (n_blocks - 1) * 128

    nc = bass.Bass("TRN2", target_bir_lowering=False, debug=False, num_devices=8,
                   dynamic_dma_scratch_size=32768)

    xT = nc.dram_tensor("xT", [128, KCH, N_PAD], BF16, kind="ExternalInput")
    wcat = nc.dram_tensor("wcat", [128, KCH, TCOLS], BF16, kind="ExternalInput")
    bias_cols = FOUT if mode == "elu" else C
    bias_in = nc.dram_tensor("bias", [128, bias_cols], F32, kind="ExternalInput")
    srcidx_in = nc.dram_tensor("srcidx", [128, NT * 8], I16, kind="ExternalInput")
    s01_in = nc.dram_tensor("s01b", [128, NT * 128], U8, kind="ExternalInput")
    s01T_in = nc.dram_tensor("s01Tb", [128, NT * 128], U8, kind="ExternalInput")
    adrow_in = nc.dram_tensor("adrow", [128, n_blocks], I32, kind="ExternalInput")
    if mode == "elu":
        out_d = nc.dram_tensor("out", [CORE_ROWS, FOUT], BF16, kind="ExternalOutput")
    else:
        out_d = nc.dram_tensor("out", [CORE_ROWS, C], F32, kind="ExternalOutput")
    table = nc.dram_tensor("table", [N_PAD, ROWB], U8)

    ST = 8                       # node tiles per staging buffer / table-write DMA
    CH = ST * 128                # xT chunk columns
    n_ch = (N_PAD + CH - 1) // CH

    with tile.TileContext(nc) as tc:
        with (
            tc.tile_pool(name="const", bufs=1) as kpool,
            tc.tile_pool(name="xchunk", bufs=4) as xpool,
            tc.tile_pool(name="stage", bufs=4) as stpool,
            tc.tile_pool(name="dpsum", bufs=2, space="PSUM") as dppool,
            tc.tile_pool(name="g", bufs=5) as gpool,
            tc.tile_pool(name="s01", bufs=5) as spool,
            tc.tile_pool(name="ee", bufs=4) as eepool,
            tc.tile_pool(name="mp", bufs=3) as mppool,
            tc.tile_pool(name="upsum", bufs=2, space="PSUM") as uppool,
            tc.tile_pool(name="epi", bufs=4) as epool,
        ):
            nc.gpsimd.load_library(library_config.mlp)
            # one Pool register per distinct gather index count (read-only after)
            nidx_reg = {}
            cap_counts = set()
            for t in list(T_lo) + list(T_hi):
                q = t
                while q > 8:
                    cap_counts.add(8)
                    q -= 8
                if q:
                    cap_counts.add(q)
            for v in sorted({t * 128 for t in cap_counts}):
                r = nc.gpsimd.alloc_register(f"nidx{v}")
                nc.gpsimd.reg_mov(r, v)
                nidx_reg[v] = r
            # ---- constants ----
            wcat_sb = kpool.tile([128, KCH * TCOLS], BF16)
            nc.sync.dma_start(out=wcat_sb[:], in_=wcat[:].rearrange("p k c -> p (k c)"))
            bias_sb = kpool.tile([128, bias_cols], F32)
            nc.sync.dma_start(out=bias_sb[:], in_=bias_in[:])
            srcidx_sb = kpool.tile([128, NT * 8], I16)
            nc.sync.dma_start(out=srcidx_sb[:], in_=srcidx_in[:])
            adrow_sb = kpool.tile([128, n_blocks], I32)
            nc.sync.dma_start(out=adrow_sb[:], in_=adrow_in[:])

            # ---- dense phase: table[N_PAD] rows = [x @ Wcat] packed ----
            for ci in range(n_ch if 'dense' not in ablate else 0):
                c0 = ci * CH
                ccols = min(CH, N_PAD - c0)
                nt_ch = ccols // 128
                xc = xpool.tile([128, KCH * CH], BF16, tag="xc")
                nc.sync.dma_start(
                    out=mkap(xc[:], [[CH, KCH], [1, ccols]]),
                    in_=xT[:, :, c0:c0 + ccols],
                )
                st = stpool.tile([128, ST * ROWB], U8, tag="st")
                st_f8 = st.bitcast(F8)
                st_bf = st.bitcast(BF16)
                for tl in range(0, nt_ch, 2):
                    npair = min(2, nt_ch - tl)
                    psum = dppool.tile([128, 1024], F32, tag="dp")
                    for half in range(npair):
                        for k in range(KCH):
                            nc.tensor.matmul(
                                psum[:, half * 512: half * 512 + TCOLS],
                                lhsT=xc[:, k * CH + (tl + half) * 128:
                                        k * CH + (tl + half) * 128 + 128],
                                rhs=wcat_sb[:, k * TCOLS:(k + 1) * TCOLS],
                                start=(k == 0),
                                stop=(k == KCH - 1),
                            )
                    nc.scalar.activation(
                        mkap(st_bf[:], [[ROWB2, npair], [1, NBF + 16]],
                             elem_offset=tl * ROWB2),
                        mkap(psum[:], [[512, npair], [1, NBF + 16]]),
                        ACTF.Copy)
                    if NF8:
                        nc.vector.tensor_copy(
                            out=mkap(st_f8[:], [[ROWB, npair], [1, NF8]],
                                     elem_offset=tl * ROWB + F8B),
                            in_=mkap(psum[:], [[512, npair], [1, NF8]],
                                     elem_offset=NBF + 16))
                nc.sync.dma_start(
                    out=table[c0:c0 + ccols, :].rearrange("(a p) c -> p a c", p=128),
                    in_=mkap(st[:], [[ROWB, nt_ch], [1, ROWB]]),
                )

            # ---- edge phase (software-pipelined emission) ----
            state = {}

            def S0(b):
                Tl, Th, T = int(T_lo[b]), int(T_hi[b]), T_all[b]
                o8 = int(toff[b]) * 8
                g = gpool.tile([128, T * ROWB], U8, tag="g")
                GCAP = 8    # max tiles per gather (SWDGE desc ring headroom)
                def emit_gathers(tbl, n_t, goff, ioff):
                    for q0 in range(0, n_t, GCAP):
                        qn = min(GCAP, n_t - q0)
                        nc.gpsimd.dma_gather(
                            mkap(g[:], [[ROWB, qn], [1, ROWB]],
                                 elem_offset=(goff + q0) * ROWB),
                            tbl,
                            srcidx_sb[:, ioff + q0 * 8: ioff + (q0 + qn) * 8],
                            qn * 128, nidx_reg[qn * 128], ROWB,
                        )
                if Tl and 'gather' not in ablate:
                    emit_gathers(table[0:SPLIT, :], Tl, 0, o8)
                if Th and 'gather' not in ablate:
                    emit_gathers(table[SPLIT:N_PAD, :], Th, Tl, o8 + Tl * 8)
                s01u = spool.tile([128, T * 128], U8, tag="s01")
                nc.sync.dma_start(
                    out=s01u[:],
                    in_=s01_in[:, int(toff[b]) * 128: (int(toff[b]) + T) * 128])
                s01Tu = spool.tile([128, T * 128], U8, tag="s01T")
                nc.sync.dma_start(
                    out=s01Tu[:],
                    in_=s01T_in[:, int(toff[b]) * 128: (int(toff[b]) + T) * 128])
                adrow = gpool.tile([128, ROWB], U8, tag="adrow")
                nc.gpsimd.indirect_dma_start(
                    out=adrow[:], out_offset=None, in_=table[:],
                    in_offset=bass.IndirectOffsetOnAxis(ap=adrow_sb[:, b:b + 1], axis=0),
                )
                state[b] = dict(g=g, s01u=s01u, s01Tu=s01Tu, adrow=adrow)

            def S1(b):
                T = T_all[b]
                st_ = state[b]
                g_bf = st_["g"].bitcast(BF16)
                s01T = st_["s01Tu"].bitcast(F8)
                adrow_bf = st_["adrow"].bitcast(BF16)
                adp = uppool.tile([128, T * H], F32, tag="adp")
                for t in range(T if 'att' not in ablate else 0):
                    nc.tensor.matmul(
                        adp[:, t * H:(t + 1) * H],
                        lhsT=s01T[:, t * 128:(t + 1) * 128],
                        rhs=adrow_bf[:, AS2 + H: AS2 + 2 * H],
                        start=True, stop=True,
                    )
                if 'att' in ablate:
                    return
                s_f = eepool.tile([128, T * H], F32, tag="sf")
                nc.vector.tensor_tensor(
                    out=s_f[:],
                    in0=mkap(g_bf[:], [[ROWB2, T], [1, H]], elem_offset=AS2),
                    in1=adp[:],
                    op=ALU.add,
                )
                s2 = eepool.tile([128, T * H], F32, tag="s2")
                nc.vector.tensor_scalar_mul(out=s2[:], in0=s_f[:], scalar1=0.2)
                nc.vector.tensor_tensor(out=s_f[:], in0=s_f[:], in1=s2[:], op=ALU.max)
                ee = eepool.tile([128, T * H], BF16, tag="ee")
                nc.scalar.activation(ee[:], s_f[:], ACTF.Exp)
                state[b]["ee"] = ee

            def S2(b):
                T = T_all[b]
                st_ = state[b]
                if 'att' in ablate:
                    return
                g_bf = st_["g"].bitcast(BF16)
                g_f8 = st_["g"].bitcast(F8)
                s01 = st_["s01u"].bitcast(F8)
                ee = st_["ee"]
                mp = mppool.tile([128, T * MCOLS], BF16, tag="mp")
                if 'mp' not in ablate:
                    nc.vector.tensor_tensor(
                        out=mkap(mp[:], [[MCOLS, T], [1, NBF]]),
                        in0=mkap(g_bf[:], [[ROWB2, T], [1, NBF]]),
                        in1=mkap(ee[:], [[H, T], [0, NBF // H], [1, H]]),
                        op=ALU.mult,
                    )
                if NF8 and 'mp' not in ablate:
                    nc.vector.tensor_tensor(
                        out=mkap(mp[:], [[MCOLS, T], [1, NF8]], elem_offset=NBF),
                        in0=mkap(g_f8[:], [[ROWB, T], [1, NF8]], elem_offset=F8B),
                        in1=mkap(ee[:], [[H, T], [0, NF8 // H], [1, H]]),
                        op=ALU.mult,
                    )
                nc.vector.tensor_copy(
                    out=mkap(mp[:], [[MCOLS, T], [1, H]], elem_offset=FOUT),
                    in_=ee[:],
                )
                up = uppool.tile([128, MCOLS], F32, tag="up")
                for t in range(T if 'up' not in ablate else 0):
                    nc.tensor.matmul(
                        up[:],
                        lhsT=s01[:, t * 128:(t + 1) * 128],
                        rhs=mp[:, t * MCOLS:(t + 1) * MCOLS],
                        start=(t == 0),
                        stop=(t == T - 1),
                    )
                state[b]["up"] = up

            def S3(b):
                if 'epi' in ablate or 'att' in ablate:
                    state.pop(b, None)
                    return
                brows = 128 if b < n_blocks - 1 else last_blk_rows
                up = state[b]["up"]
                rec = epool.tile([128, H], F32, tag="rec")
                nc.vector.reciprocal(out=rec[:brows], in_=up[:brows, FOUT:FOUT + H])
                u = epool.tile([128, FOUT], F32, tag="u")
                nc.vector.tensor_tensor(
                    out=u[:brows], in0=up[:brows, 0:FOUT],
                    in1=mkap(rec[:brows], [[0, FOUT // H], [1, H]]),
                    op=ALU.mult,
                )
                if with_bias:
                    nc.vector.tensor_tensor(out=u[:brows], in0=u[:brows],
                                            in1=bias_sb[:brows], op=ALU.add)
                if mode == "elu":
                    # out = ELU(u)+1 = relu(u) + min(exp(u), 1); host subtracts 1
                    ex = epool.tile([128, FOUT], F32, tag="ex")
                    nc.scalar.activation(ex[:brows], u[:brows], ACTF.Exp)
                    r = epool.tile([128, FOUT], F32, tag="r")
                    nc.vector.tensor_scalar_max(out=r[:brows], in0=u[:brows], scalar1=0.0)
                    em = epool.tile([128, FOUT], F32, tag="em")
                    nc.vector.tensor_scalar_min(out=em[:brows], in0=ex[:brows], scalar1=1.0)
                    ob = epool.tile([128, FOUT], BF16, tag="ob")
                    nc.vector.tensor_tensor(out=ob[:brows], in0=r[:brows], in1=em[:brows],
                                            op=ALU.add)
                    if 'epiout' not in ablate:
                        nc.sync.dma_start(out=out_d[b * 128: b * 128 + brows, :], in_=ob[:brows])
                else:
                    m1 = epool.tile([128, FOUT // 2], F32, tag="m1")
                    nc.vector.tensor_tensor(
                        out=m1[:brows],
                        in0=mkap(u[:brows], [[H, C], [1, H // 2]]),
                        in1=mkap(u[:brows], [[H, C], [1, H // 2]], elem_offset=H // 2),
                        op=ALU.add)
                    m2 = epool.tile([128, FOUT // 4], F32, tag="m2")
                    nc.vector.tensor_tensor(
                        out=m2[:brows],
                        in0=mkap(m1[:brows], [[H // 2, C], [1, H // 4]]),
                        in1=mkap(m1[:brows], [[H // 2, C], [1, H // 4]], elem_offset=H // 4),
                        op=ALU.add)
                    m3 = epool.tile([128, C], F32, tag="m3")
                    nc.vector.tensor_tensor(
                        out=m3[:brows],
                        in0=mkap(m2[:brows], [[2, C]]),
                        in1=mkap(m2[:brows], [[2, C]], elem_offset=1),
                        op=ALU.add)
                    zb = epool.tile([128, C], F32, tag="zb")
                    nc.vector.tensor_scalar_mul(out=zb[:brows], in0=m3[:brows], scalar1=1.0 / H)
                    if with_bias:
                        nc.vector.tensor_tensor(out=zb[:brows], in0=zb[:brows],
                                                in1=bias_sb[:brows], op=ALU.add)
                    exs = epool.tile([128, C], F32, tag="exs")
                    sms = epool.tile([128, 1], F32, tag="sms")
                    nc.scalar.activation(exs[:brows], zb[:brows], ACTF.Exp,
                                         accum_out=sms[:brows])
                    lg = epool.tile([128, 1], F32, tag="lg")
                    nc.scalar.activation(lg[:brows], sms[:brows], ACTF.Ln)
                    outt = epool.tile([128, C], F32, tag="outt")
                    nc.vector.tensor_scalar(out=outt[:brows], in0=zb[:brows],
                                            scalar1=lg[:brows, 0:1], scalar2=None,
                                            op0=ALU.subtract)
                    if 'epiout' not in ablate:
                        nc.sync.dma_start(out=out_d[b * 128: b * 128 + brows, :], in_=outt[:brows])
                state.pop(b, None)

            for slot in range(n_blocks + 3):
                if slot < n_blocks:
                    S0(slot)
                if 1 <= slot < n_blocks + 1:
                    S1(slot - 1)
                if 2 <= slot < n_blocks + 2:
                    S2(slot - 2)
                if slot >= 3:
                    S3(slot - 3)
    return nc


# ---------------- host side ----------------

def fold_weights(W, a_src, a_dst, H, C, perm, NBF):
    """Wcat [FIN, FOUT + 2H] f32: [bf16 h-part | Wa_src | Wa_dst | f8 h-part]."""
    WT = np.asarray(W, np.float32).T.copy()           # [FIN, H*C] logical (h,c)
    FIN = WT.shape[0]
    W3 = WT.reshape(FIN, H, C)
    Wa_s = np.einsum('fhc,hc->fh', W3, np.asarray(a_src, np.float32))
    Wa_d = np.einsum('fhc,hc->fh', W3, np.asarray(a_dst, np.float32))
    WP = WT[:, perm]
    return np.concatenate([WP[:, :NBF], Wa_s, Wa_d, WP[:, NBF:]], axis=1)


def col_perm(H, C, NBF):
    """Table h-column order: (c,h)-major (h fastest), bf16 part = first NBF//H
    c-groups. Returns perm with perm[i] = logical col (h*C+c) at table pos i."""
    idx = np.arange(H * C).reshape(H, C).T.reshape(-1)   # (c,h)-major
    return idx


def pack_kdim(M):
    """[FIN, COLS] -> [128, KCH, COLS]: row k*128+p -> [p, k]."""
    FIN, COLS = M.shape
    KCH = FIN // 128
    return np.ascontiguousarray(M.reshape(KCH, 128, COLS).transpose(1, 0, 2))


def wrap16(idx):
    """[n] int -> wrapped int16 [16, n//16]: idx[i] at [i%16, i//16]."""
    n = idx.shape[0]
    assert n % 16 == 0
    return np.ascontiguousarray(idx.reshape(n // 16, 16).T.astype(np.int16))


def route_edges(src, dst):
    """Per-core gather indices/dst locations, per-block lo/hi tile counts."""
    n_blocks = (CORE_ROWS + 127) // 128
    core_of = dst // CORE_ROWS
    per_core = []
    nlo = np.zeros((N_CORES, n_blocks), np.int64)
    nhi = np.zeros((N_CORES, n_blocks), np.int64)
    for c in range(N_CORES):
        m = core_of == c
        s_c = src[m]
        dl = dst[m] - c * CORE_ROWS
        blk = dl // 128
        loc = dl - blk * 128
        hi = (s_c >= SPLIT).astype(np.int64)
        order = np.lexsort((s_c, hi, blk))
        s_c, blk, loc, hi = s_c[order], blk[order], loc[order], hi[order]
        per_core.append((s_c, blk, loc, hi))
        for b in range(n_blocks):
            mb_ = blk == b
            nlo[c, b] = int((hi[mb_] == 0).sum())
            nhi[c, b] = int(hi[mb_].sum())
    T_lo = [int(x) for x in np.ceil(nlo.max(axis=0) / 128.0).astype(int)]
    T_hi = [int(x) for x in np.ceil(nhi.max(axis=0) / 128.0).astype(int)]
    T_all = [T_lo[b] + T_hi[b] for b in range(n_blocks)]
    toff = np.concatenate([[0], np.cumsum(T_all)]).astype(int)
    NT = int(toff[-1])
    out = []
    for c in range(N_CORES):
        s_c, blk, loc, hi = per_core[c]
        sidx16 = np.zeros((128, NT * 8), np.int16)
        dloc = np.full((NT * 128,), -1.0, np.float32)   # position-major (t*128+j)
        for b in range(n_blocks):
            mb_ = blk == b
            sb, lb, hb = s_c[mb_], loc[mb_], hi[mb_]
            ml, mh = hb == 0, hb == 1
            ilo = np.zeros(T_lo[b] * 128, np.int64)
            ilo[:ml.sum()] = sb[ml]
            ihi = np.zeros(T_hi[b] * 128, np.int64)
            ihi[:mh.sum()] = sb[mh] - SPLIT
            o8 = int(toff[b]) * 8
            if T_lo[b]:
                sidx16[:, o8: o8 + T_lo[b] * 8] = np.tile(wrap16(ilo), (8, 1))
            if T_hi[b]:
                sidx16[:, o8 + T_lo[b] * 8: o8 + T_all[b] * 8] = np.tile(wrap16(ihi), (8, 1))
            p0 = int(toff[b]) * 128
            dloc[p0: p0 + ml.sum()] = lb[ml]
            p1 = p0 + T_lo[b] * 128
            dloc[p1: p1 + mh.sum()] = lb[mh]
        # fp8 selection bitmaps (0x38 = 1.0 in e4m3)
        dl = dloc.reshape(NT, 128)                                # [tile, j]
        d128 = np.arange(128, dtype=np.float32)
        s01T_b = (dl.reshape(1, NT * 128) == d128[:, None]).astype(np.uint8) * 0x38
        s01_b = np.ascontiguousarray(
            (dl[:, :, None] == d128).transpose(1, 0, 2).reshape(128, NT * 128)
        ).astype(np.uint8) * 0x38
        base = c * CORE_ROWS
        adrow = np.zeros((128, n_blocks), np.int32)
        for b in range(n_blocks):
            adrow[:, b] = base + b * 128 + np.arange(128)
        out.append({"srcidx": sidx16, "s01b": s01_b,
                    "s01Tb": np.ascontiguousarray(s01T_b), "adrow": adrow})
    return T_lo, T_hi, out


MAX_WAITS = 1


def fix_excess_waits(nc):
    """Post-process BIR JSON: any instruction with >MAX_WAITS sem-waits gets
    preceding Nop instructions carrying the excess waits (same engine, in-order).
    Monkeypatches nc.to_json_bytes to return the fixed JSON."""
    raw = nc.to_json_bytes()
    d = json.loads(raw)
    n_fix = 0
    for f in d["functions"]:
        for bb in f["blocks"]:
            out = []
            for inst in bb["instructions"]:
                si = inst.get("sync_info")
                waits = (si or {}).get("on_wait") or []
                if len(waits) > MAX_WAITS:
                    extra = waits[:-MAX_WAITS]
                    keep = waits[-MAX_WAITS:]
                    for ci in range(0, len(extra), MAX_WAITS):
                        chunk = extra[ci:ci + MAX_WAITS]
                        n_fix += 1
                        out.append({
                            "debug": inst.get("debug", 0),
                            "engine": inst["engine"],
                            "ins": [],
                            "is_reset_sema": False,
                            "name": f"{inst['name']}-wfix{ci}",
                            "opcode": "EventSemaphore",
                            "outs": [],
                            "sync_info": {"on_update": [], "on_wait": chunk},
                        })
                    si["on_wait"] = keep
                out.append(inst)
            bb["instructions"] = out
    fixed = json.dumps(d).encode()
    nc.to_json_bytes = lambda: fixed
    return n_fix


# ---------------- top-level kernel ----------------

_CACHE = {}


def _get_program(key, builder):
    if key not in _CACHE:
        nc = builder()
        from concourse.library_overlay import lower_extended_insts
        lower_extended_insts(nc)
        fix_excess_waits(nc)
        _CACHE[key] = nc
    return _CACHE[key]


NBF1 = 224   # layer-1 bf16 h columns (of 256); rest fp8
NBF2 = 160   # layer-2 bf16 h columns (of 320); rest fp8


def kernel(x, edge_index, W1, a_src1, a_dst1, b1, W2, a_src2, a_dst2, b2):
    from concourse.bass_utils import run_bass_kernel_spmd

    x = np.asarray(x, np.float32)
    ei = np.asarray(edge_index)
    N = N_NODES
    src = np.concatenate([ei[0], np.arange(N)]).astype(np.int64)
    dst = np.concatenate([ei[1], np.arange(N)]).astype(np.int64)
    T_lo, T_hi, routed = route_edges(src, dst)
    tkey = (tuple(T_lo), tuple(T_hi))

    def xpad_T(a):
        ap = np.zeros((N_PAD, a.shape[1]), np.float32)
        ap[:a.shape[0]] = a
        return np.ascontiguousarray(ap.T)

    # ---- layer 1 ----
    H1, C1 = 8, 32
    perm1 = col_perm(H1, C1, NBF1)
    wb1 = bool(np.any(np.asarray(b1)))
    Wcat1 = fold_weights(W1, a_src1, a_dst1, H1, C1, perm1, NBF1)
    nc1 = _get_program(("l1", NBF1, wb1) + tkey, lambda: build_gat_layer(
        128, H1, C1, NBF1, T_lo, T_hi, "elu", wb1))
    com1 = {
        "xT": pack_kdim(xpad_T(x)).astype(bf16),
        "wcat": pack_kdim(Wcat1).astype(bf16),
        "bias": np.tile(np.asarray(b1, np.float32)[perm1][None, :], (128, 1)),
    }
    in_maps1 = [dict(com1, **routed[c]) for c in range(N_CORES)]
    res1 = run_bass_kernel_spmd(nc1, in_maps1, list(range(N_CORES)))
    h1p = np.concatenate([np.asarray(res1.results[c]["out"]) for c in range(N_CORES)], axis=0)
    # undo ELU+1 shift and the (c,h)-major permutation -> logical (h,c) cols
    h1 = np.zeros((N, H1 * C1), np.float32)
    h1[:, perm1] = np.float32(h1p) - 1.0

    # ---- layer 2 ----
    H2, C2 = 8, 40
    perm2 = col_perm(H2, C2, NBF2)
    wb2 = bool(np.any(np.asarray(b2)))
    Wcat2 = fold_weights(W2, a_src2, a_dst2, H2, C2, perm2, NBF2)
    nc2 = _get_program(("l2", NBF2, wb2) + tkey, lambda: build_gat_layer(
        256, H2, C2, NBF2, T_lo, T_hi, "mean_lsm", wb2))
    com2 = {
        "xT": pack_kdim(xpad_T(h1)).astype(bf16),
        "wcat": pack_kdim(Wcat2).astype(bf16),
        "bias": np.tile(np.asarray(b2, np.float32)[None, :], (128, 1)),
    }
    in_maps2 = [dict(com2, **routed[c]) for c in range(N_CORES)]
    res2 = run_bass_kernel_spmd(nc2, in_maps2, list(range(N_CORES)))
    out = np.concatenate([np.asarray(res2.results[c]["out"]) for c in range(N_CORES)], axis=0)
    return out.astype(np.float32)
